# revision 3
# baseline (speedup 1.0000x reference)
"""GraphConv 2-layer GNN on 8 Trainium2 NeuronCores — fused-slab version.

Layer 1 fuses the segment-sum into the W1_rel matmul: the host lays out the
edge payload as columns of x.T (features on partitions, one column per edge),
grouped into degree-sorted prefix "slabs" (slab j = the j-th neighbor
contribution, which by descending-degree order covers a contiguous prefix of
node ranks).  An accumulating PE matmul chain with stationary W1_rel.T
computes W1_rel @ segment_sum(x[src]) directly in PSUM, per 512-node stripe,
with the W1_root @ x.T term as one extra accumulating matmul.  relu+bias
yields h.T, and a packed [128,80] weight computes both y2.T = W2_rel@h.T
(the 40-wide layer-2 edge features) and r2.T = W2_root@h.T in one matmul.

The host then gathers y2[src] into a windowed layout and layer 2 reduces it
with an identity-matmul chain (identity stationary the whole launch), adds
the root term, and applies log-softmax along the free axis (exp/ln phases
batched so the activation table loads only twice).

Payloads are fp8e4m3 (PSUM accumulation stays exact f32; only quantization
error enters).  Host work is layout/permutation prep only.
"""
import sys
sys.path.insert(0, "/opt/trn_rl_repo")
import numpy as np
import ml_dtypes

import concourse.bacc as bacc
import concourse.mybir as mybir
import concourse.tile as tile
from concourse.bass_utils import run_bass_kernel_spmd
from concourse.masks import make_identity

BF16 = ml_dtypes.bfloat16
FP8 = ml_dtypes.float8_e4m3
N, E, F, H, C = 100000, 1600000, 128, 128, 40
NCORES = 8
OWN = N // NCORES          # 12500 dst nodes per core
P = 128
STRIPE = 512               # nodes per PSUM stripe (launch 1)
NSTR = (OWN + STRIPE - 1) // STRIPE    # 25
OWNP = NSTR * STRIPE       # 12800
G2 = 10                    # windows per group (launch 2)
NGRP = OWNP // (G2 * P)    # 10 groups of 1280 nodes
GC = G2 * C                # 400
ALIGN = 16                 # pay1 piece width alignment (cols)

PAY2_FP8 = True
Y2O_FP8 = True             # write y2 as fp8 from launch 1 directly

BF = mybir.dt.bfloat16
F8 = mybir.dt.float8e4
F32 = mybir.dt.float32
P2DT = F8 if PAY2_FP8 else BF
P2NP = FP8 if PAY2_FP8 else BF16
Y2DT = F8 if Y2O_FP8 else BF
Y2NP = FP8 if Y2O_FP8 else BF16


class Sched:
    pass


def _prep_graph(edge_index):
    """Shared (cross-core) schedule + per-core edge->column maps."""
    src = np.asarray(edge_index[0], dtype=np.int64)
    dst = np.asarray(edge_index[1], dtype=np.int64)
    deg = np.bincount(dst, minlength=N)

    orders = []           # per core: global node ids in degree-desc order
    degs = np.zeros((NCORES, OWNP), np.int64)
    for c in range(NCORES):
        ids = np.arange(c * OWN, (c + 1) * OWN)
        o = ids[np.argsort(-deg[ids], kind="stable")]
        orders.append(o)
        degs[c, :OWN] = deg[o]

    Jmax = int(degs.max())
    # n[c, j] = number of ranks on core c with deg > j
    n = (degs[:, None, :] > np.arange(Jmax)[None, :, None]).sum(2)  # [8, Jmax]

    # launch-1 piece widths W[s][j], stripe-major then j; W[s][0] forced 512
    Ws = []               # per stripe: list of widths (multiples of ALIGN)
    for s in range(NSTR):
        a = s * STRIPE
        w = np.clip(np.minimum(n, (s + 1) * STRIPE) - a, 0, STRIPE).max(0)  # [Jmax]
        w = w[w > 0]
        w = np.minimum((w + ALIGN - 1) // ALIGN * ALIGN, STRIPE)
        lst = w.tolist()
        if not lst:
            lst = [STRIPE]
        lst[0] = STRIPE
        Ws.append(lst)
    stripe_cols = [int(sum(w)) for w in Ws]
    stripe_off = np.zeros(NSTR + 1, np.int64)
    stripe_off[1:] = np.cumsum(stripe_cols)
    C1 = int(stripe_off[-1])

    # per-(s,j) global column base
    CB = np.full((NSTR, Jmax + 1), -1, np.int64)
    for s in range(NSTR):
        loc = 0
        for j, w in enumerate(Ws[s]):
            CB[s, j] = stripe_off[s] + loc
            loc += w

    # launch-2 group depths
    Dg = np.maximum(degs[:, ::G2 * P].max(0), 1).astype(np.int64)  # [NGRP]
    off2 = np.zeros(NGRP + 1, np.int64)
    off2[1:] = np.cumsum(Dg * G2)          # in 40-col units
    C2_40 = int(off2[-1])

    # per-core edge -> (col1, p2/col40) maps
    core = dst // OWN
    cmaps = []
    for c in range(NCORES):
        rank_of = np.empty(OWN, np.int64)
        rank_of[orders[c] - c * OWN] = np.arange(OWN)
        m = core == c
        s_c, d_c = src[m], dst[m]
        r = rank_of[d_c - c * OWN]
        perm = np.argsort(r, kind="stable")
        r_s = r[perm]
        s_s = s_c[perm]
        first = np.searchsorted(r_s, r_s)
        j = np.arange(len(r_s)) - first      # occurrence index within dst
        col1 = CB[r_s >> 9, j] + (r_s & (STRIPE - 1))
        g = r_s // (G2 * P)
        col40 = off2[g] + j * G2 + ((r_s >> 7) % G2)
        p2 = r_s & (P - 1)
        cmaps.append((s_s, col1, p2, col40))

    sc = Sched()
    sc.orders, sc.degs = orders, degs
    sc.Ws, sc.stripe_off, sc.C1 = Ws, stripe_off, C1
    sc.Dg, sc.off2, sc.C2_40 = Dg, off2, C2_40
    sc.cmaps = cmaps
    return sc


def _inputs1(sc, x, W1_rel, b1, W1_root, W2_rel, W2_root):
    x8 = np.asarray(x, np.float32).astype(FP8)
    xb = np.asarray(x, np.float32).astype(BF16)
    w1relT = np.ascontiguousarray(np.asarray(W1_rel, np.float32).T).astype(BF16)
    w1rootT = np.ascontiguousarray(np.asarray(W1_root, np.float32).T).astype(BF16)
    w2pT = np.zeros((H, 64 + C), np.float32)
    w2pT[:, :C] = np.asarray(W2_rel, np.float32).T
    w2pT[:, 64:] = np.asarray(W2_root, np.float32).T
    w2pT = w2pT.astype(BF16)
    b1v = np.zeros((P, 1), np.float32)
    b1v[:H, 0] = np.asarray(b1, np.float32)
    in_maps = []
    for c in range(NCORES):
        s_s, col1, _, _ = sc.cmaps[c]
        pay_cm = np.zeros((sc.C1, F), FP8)
        pay_cm[col1] = x8[s_s]
        pay1 = np.ascontiguousarray(pay_cm.T)
        xT = np.zeros((F, OWNP), BF16)
        xT[:, :OWN] = xb[sc.orders[c]].T
        in_maps.append({"pay1": pay1, "xT": xT, "w1relT": w1relT,
                        "w1rootT": w1rootT, "w2pT": w2pT, "b1v": b1v})
    return in_maps


def _inputs2(sc, y2s, r2s, b2):
    """Launch-2 inputs from launch-1 outputs y2o [C,OWNP] / r2o [C,OWNP]."""
    b2f = np.asarray(b2, np.float32)
    y2g = np.zeros((N, C), P2NP)
    for c in range(NCORES):
        y2g[sc.orders[c]] = y2s[c][:, :OWN].T.astype(P2NP)
    in_maps = []
    for c in range(NCORES):
        s_s, _, p2, col40 = sc.cmaps[c]
        pay = np.zeros((P, sc.C2_40, C), P2NP)
        pay[p2, col40] = y2g[s_s]
        r2 = r2s[c].astype(np.float32).T + b2f               # [OWNP, 40]
        r2b = np.ascontiguousarray(
            r2.reshape(NGRP, G2, P, C).transpose(2, 0, 1, 3)).astype(BF16)
        in_maps.append({"pay2": pay.reshape(P, -1), "r2b": r2b.reshape(P, -1)})
    return in_maps


def _build1(sc, R=1):
    nc = bacc.Bacc()
    pay1 = nc.declare_dram_parameter("pay1", [P, sc.C1], F8, isOutput=False)
    xT = nc.declare_dram_parameter("xT", [F, OWNP], BF, isOutput=False)
    w1relT = nc.declare_dram_parameter("w1relT", [F, H], BF, isOutput=False)
    w1rootT = nc.declare_dram_parameter("w1rootT", [F, H], BF, isOutput=False)
    w2pT = nc.declare_dram_parameter("w2pT", [H, 64 + C], BF, isOutput=False)
    b1v = nc.declare_dram_parameter("b1v", [P, 1], F32, isOutput=False)
    y2o = nc.declare_dram_parameter("y2o", [C, OWNP], Y2DT, isOutput=True)
    r2o = nc.declare_dram_parameter("r2o", [C, OWNP], BF, isOutput=True)

    with tile.TileContext(nc) as tc:
        with (
            tc.tile_pool(name="const", bufs=1) as cpool,
            tc.tile_pool(name="stream", bufs=3) as spool,
            tc.tile_pool(name="h", bufs=3) as hpool,
            tc.tile_pool(name="yrst", bufs=1) as ypool,
            tc.tile_pool(name="ph", bufs=2, space="PSUM") as php,
            tc.tile_pool(name="py", bufs=2, space="PSUM") as pyp,
        ):
            w1rel_t = cpool.tile([F, H], BF)
            nc.sync.dma_start(out=w1rel_t[:], in_=w1relT[:])
            w1root_t = cpool.tile([F, H], BF)
            nc.sync.dma_start(out=w1root_t[:], in_=w1rootT[:])
            w2p_t = cpool.tile([H, 64 + C], BF)
            nc.sync.dma_start(out=w2p_t[:], in_=w2pT[:])
            b1_t = cpool.tile([P, 1], F32)
            nc.sync.dma_start(out=b1_t[:], in_=b1v[:])
            xT_t = cpool.tile([F, OWNP], BF)
            nc.sync.dma_start(out=xT_t[:], in_=xT[:])

            def body(_iv=None):
                y2_sb = ypool.tile([C, OWNP], Y2DT, tag="y2")
                r2_sb = ypool.tile([C, OWNP], BF, tag="r2")
                order = list(range(NSTR))[::-1]
                for si, s in enumerate(order):
                    a = s * STRIPE
                    c0, c1 = int(sc.stripe_off[s]), int(sc.stripe_off[s + 1])
                    st = spool.tile([P, c1 - c0], F8, tag="pay")
                    nc.sync.dma_start(out=st[:], in_=pay1[:, c0:c1])
                    ph = php.tile([P, STRIPE], F32, tag="ph")
                    loc = 0
                    ws = sc.Ws[s]
                    for j, w in enumerate(ws):
                        nc.tensor.matmul(
                            out=ph[:, :w], lhsT=w1rel_t[:],
                            rhs=st[:, loc:loc + w],
                            start=(j == 0), stop=False,
                        )
                        loc += w
                    nc.tensor.matmul(
                        out=ph[:], lhsT=w1root_t[:],
                        rhs=xT_t[:, a:a + STRIPE],
                        start=False, stop=True,
                    )
                    ht = hpool.tile([P, STRIPE], BF, tag="ht")
                    nc.scalar.activation(
                        out=ht[:], in_=ph[:],
                        func=mybir.ActivationFunctionType.Relu,
                        bias=b1_t[:, :1], scale=1.0,
                    )
                    py = pyp.tile([64 + C, STRIPE], F32, tag="py")
                    nc.tensor.matmul(out=py[:], lhsT=w2p_t[:], rhs=ht[:],
                                     start=True, stop=True)
                    nc.vector.tensor_copy(out=y2_sb[:, a:a + STRIPE],
                                          in_=py[:C, :])
                    nc.vector.tensor_copy(out=r2_sb[:, a:a + STRIPE],
                                          in_=py[64:64 + C, :])
                    if si % 5 == 4:
                        lo = min(order[si - 4], s) * STRIPE
                        hi = max(order[si - 4], s) * STRIPE + STRIPE
                        nc.sync.dma_start(out=y2o[:, lo:hi],
                                          in_=y2_sb[:, lo:hi])
                        nc.sync.dma_start(out=r2o[:, lo:hi],
                                          in_=r2_sb[:, lo:hi])

            if R > 1:
                with tc.For_i(0, R, 1):
                    body()
            else:
                body()
    nc.finalize()
    return nc


def _build2(sc, R=1):
    nc = bacc.Bacc()
    pay2 = nc.declare_dram_parameter("pay2", [P, sc.C2_40 * C], P2DT, isOutput=False)
    r2b = nc.declare_dram_parameter("r2b", [P, NGRP * G2 * C], BF, isOutput=False)
    out = nc.declare_dram_parameter("out", [P, NGRP * G2 * C], BF, isOutput=True)

    with tile.TileContext(nc) as tc:
        with (
            tc.tile_pool(name="const", bufs=1) as cpool,
            tc.tile_pool(name="stream", bufs=3) as spool,
            tc.tile_pool(name="work", bufs=3) as wpool,
            tc.tile_pool(name="stash", bufs=1) as tpool,
            tc.tile_pool(name="ost", bufs=1) as opool,
            tc.tile_pool(name="ps", bufs=2, space="PSUM") as ppool,
        ):
            ident = cpool.tile([P, P], BF)
            make_identity(nc, ident[:])
            r2_t = cpool.tile([P, NGRP, G2, C], BF)
            nc.sync.dma_start(
                out=r2_t[:].rearrange("p g i c -> p (g i c)"), in_=r2b[:])

            def body(_iv=None):
                smst = tpool.tile([P, NGRP, G2, C], F32, tag="smst")
                mxst = tpool.tile([P, NGRP, G2, 1], F32, tag="mxst")
                smest = tpool.tile([P, NGRP, G2, 1], F32, tag="smest")
                for g in range(NGRP):
                    D = int(sc.Dg[g])
                    o0 = int(sc.off2[g]) * C
                    st = spool.tile([P, D * GC], P2DT, tag="pay")
                    nc.sync.dma_start(out=st[:], in_=pay2[:, o0:o0 + D * GC])
                    ps = ppool.tile([P, GC], F32, tag="agg")
                    for d in range(D):
                        nc.tensor.matmul(
                            out=ps[:], lhsT=ident[:],
                            rhs=st[:, d * GC:(d + 1) * GC],
                            start=(d == 0), stop=(d == D - 1),
                        )
                    nc.vector.tensor_add(
                        out=smst[:, g].rearrange("p i c -> p (i c)"),
                        in0=ps[:],
                        in1=r2_t[:, g].rearrange("p i c -> p (i c)"))
                    nc.vector.reduce_max(out=mxst[:, g], in_=smst[:, g],
                                         axis=mybir.AxisListType.X)
                    ex = wpool.tile([P, G2, C], F32, tag="ex")
                    nc.vector.tensor_tensor(
                        out=ex[:], in0=smst[:, g],
                        in1=mxst[:, g].to_broadcast([P, G2, C]),
                        op=mybir.AluOpType.subtract)
                    nc.scalar.activation(
                        out=ex[:], in_=ex[:],
                        func=mybir.ActivationFunctionType.Exp)
                    nc.vector.reduce_sum(out=smest[:, g], in_=ex[:],
                                         axis=mybir.AxisListType.X)
                # batched ln + final subtract (2 activation tables total)
                ls = wpool.tile([P, NGRP * G2], F32, tag="ls")
                nc.scalar.activation(
                    out=ls[:], in_=smest[:].rearrange("p g i o -> p (g i o)"),
                    func=mybir.ActivationFunctionType.Ln)
                tot = wpool.tile([P, NGRP, G2, 1], F32, tag="tot")
                nc.vector.tensor_add(
                    out=tot[:].rearrange("p g i o -> p (g i o)"),
                    in0=mxst[:].rearrange("p g i o -> p (g i o)"), in1=ls[:])
                ost = opool.tile([P, NGRP, G2, C], BF, tag="ost")
                nc.vector.tensor_tensor(
                    out=ost[:], in0=smst[:],
                    in1=tot[:].to_broadcast([P, NGRP, G2, C]),
                    op=mybir.AluOpType.subtract)
                nc.sync.dma_start(
                    out=out[:], in_=ost[:].rearrange("p g i c -> p (g i c)"))

            if R > 1:
                with tc.For_i(0, R, 1):
                    body()
            else:
                body()
    nc.finalize()
    return nc


def kernel(x, edge_index, W1_rel, b1, W1_root, W2_rel, b2, W2_root):
    sc = _prep_graph(edge_index)
    nc1 = _build1(sc)
    nc2 = _build2(sc)

    in1 = _inputs1(sc, x, W1_rel, b1, W1_root, W2_rel, W2_root)
    res1 = run_bass_kernel_spmd(nc1, in1, list(range(NCORES)))
    y2s = [res1.results[c]["y2o"] for c in range(NCORES)]
    r2s = [res1.results[c]["r2o"] for c in range(NCORES)]

    in2 = _inputs2(sc, y2s, r2s, b2)
    res2 = run_bass_kernel_spmd(nc2, in2, list(range(NCORES)))

    out = np.zeros((N, C), np.float32)
    for c in range(NCORES):
        o = res2.results[c]["out"].astype(np.float32).reshape(P, NGRP * G2, C)
        o = o.transpose(1, 0, 2).reshape(OWNP, C)[:OWN]
        out[sc.orders[c]] = o
    return out
